# revision 18
# baseline (speedup 1.0000x reference)
"""Trainium2 Bass kernel for nn_Block_85598698209846 (moe_routing).

Strategy (8 NeuronCores, SPMD single program, per-core data):
- Tokens are assigned to cores BY EXPERT (host routes via eids): core c owns
  exactly the tokens that route to expert c, sorted by (batch, position).
  MoE then needs no communication and each core loads only its expert.
- Attention: K/V are computed in contiguous position blocks (core r owns
  block r) and shared via AllGather; each core computes Q for its
  scattered-but-sorted tokens. Causality is recovered with compile-time
  column windows (shared across cores) plus small per-core uploaded masks.
  Softmax runs without max-subtraction (|scores| <= 8), matching reference.
- The input rms_norm is skipped on the Q/K path (the per-head rms_norm is
  scale-invariant, so it cancels); V and the MoE input use the real norm.
- Rope uses a host-permuted "paired" dim layout so the rotation partner
  sits on the adjacent partition: one stream_shuffle replaces 4 copies.
- All 4 GQA query heads of one kv head are interleaved into a single
  score strip (columns 4t+u), quartering matmul/exp instruction counts.
- Layout: all activations transposed [D on partitions, tokens on free].
"""
import contextlib
import numpy as np
import ml_dtypes

import concourse.bass as bass
import concourse.bacc as bacc
import concourse.tile as tile
from concourse import mybir
from concourse.bass_utils import run_bass_kernel_spmd

B, S, D = 2, 2048, 1024
NH, NKV, HD = 16, 4, 64
KVD = NKV * HD
NE, INTER = 8, 512
EPS = float(np.float32(1.1920929e-07))
NCORES = 8
KVBLK = 512          # seq rows per core in the KV phase
NKVT = S // 128      # 16 kv tiles per batch
NDT = D // 128       # 8 d-tiles
F32 = mybir.dt.float32
BF16 = mybir.dt.bfloat16
STRIP4 = 1024        # score-strip width in interleaved cols (2 PSUM banks)
TOKCAP = STRIP4 // 4
ALU = mybir.AluOpType
ACT = mybir.ActivationFunctionType
SWAP_MASK = [x for k in range(16) for x in (2 * k + 1, 2 * k)]


# ---------------------------------------------------------------- host side

def _pair_perm(nheads):
    """Output-dim permutation putting rope partners on adjacent rows."""
    p = []
    for h in range(nheads):
        for i in range(HD // 2):
            p += [64 * h + i, 64 * h + 32 + i]
    return np.array(p, dtype=np.int64)


PERMQ = _pair_perm(NH)
PERMK = _pair_perm(NKV)


def _route(eids):
    eids = np.asarray(eids).astype(np.int64)
    lists = [[np.sort(np.where(eids[b] == e)[0]) for b in range(B)]
             for e in range(NE)]
    maxn = max(len(lists[e][b]) for e in range(NE) for b in range(B))
    CB = max(64, ((maxn + 63) // 64) * 64)
    cols = np.zeros((NE, B, CB), dtype=np.int64)
    nreal = np.zeros((NE, B), dtype=np.int64)
    for e in range(NE):
        for b in range(B):
            L = lists[e][b]
            nreal[e, b] = len(L)
            if len(L):
                cols[e, b, :len(L)] = L
                cols[e, b, len(L):] = L[-1]
    return cols, nreal, CB


def _windows(cols, CB):
    Wt = np.zeros((B, NKVT), dtype=np.int64)
    Mt = np.zeros((B, NKVT), dtype=np.int64)
    for b in range(B):
        for j in range(NKVT):
            Wt[b, j] = min(int(np.searchsorted(cols[e, b], 128 * j))
                           for e in range(NE))
            Mt[b, j] = max(int(np.searchsorted(cols[e, b], 128 * j + 127))
                           for e in range(NE))
    return Wt, Mt


def _groups4(Wt, CB):
    """Per batch: split each kv tile's token window into parts of <= TOKCAP
    tokens and greedy-pack them into strips of <= STRIP4 interleaved cols.
    groups[b] = list of strips; strip = list of (j, t0, tl, ofs4)."""
    groups = []
    for b in range(B):
        parts = []
        for j in range(NKVT):
            t0 = int(Wt[b, j])
            while t0 < CB:
                tl = min(TOKCAP, CB - t0)
                parts.append((j, t0, tl))
                t0 += tl
        gs, cur, ofs = [], [], 0
        for (j, t0, tl) in parts:
            if ofs + 4 * tl > STRIP4:
                gs.append(cur)
                cur, ofs = [], 0
            cur.append((j, t0, tl, ofs))
            ofs += 4 * tl
        if cur:
            gs.append(cur)
        groups.append(gs)
    return groups


def _mask_layout(Wt, Mt, CB):
    ofs, total = {}, 0
    for b in range(B):
        for j in range(NKVT):
            if Wt[b, j] >= CB:
                continue
            mw = int(min(Mt[b, j], CB) - Wt[b, j])
            if mw <= 0:
                continue
            ofs[(b, j)] = (total, mw)
            total += mw
    return ofs, max(total, 1)


def _rope_tables_paired(positions):
    """[128, n] cos2/sin2 for paired-layout rope (2 heads per tile)."""
    inv_freq = (1.0 / 10000.0 ** (np.arange(0, HD, 2, dtype=np.float32) / HD)
                ).astype(np.float32)
    fr = np.outer(inv_freq, positions.astype(np.float32))  # [32, n]
    c, s = np.cos(fr), np.sin(fr)
    half_c = np.empty((64, fr.shape[1]), np.float32)
    half_s = np.empty((64, fr.shape[1]), np.float32)
    half_c[0::2], half_c[1::2] = c, c
    half_s[0::2], half_s[1::2] = s, -s
    cos2 = np.concatenate([half_c, half_c], axis=0)
    sin2 = np.concatenate([half_s, half_s], axis=0)
    return np.ascontiguousarray(cos2), np.ascontiguousarray(sin2)


def _vec8(v):
    return np.ascontiguousarray(np.asarray(v, np.float32).reshape(NDT, 128).T)


def _build_core_inputs(c, xm, vel, attn_scale, mlp_scale, mu_c, qg,
                       cqpT, ckpT, cv_wT, proj_wT, gate_up, down,
                       cols, CB, Wt, Mt, mofs, MW):
    f = np.float32
    bf = ml_dtypes.bfloat16
    pos = cols[c]                                     # [B, CB]
    bidx = np.repeat(np.arange(B), CB)
    sidx = pos.reshape(-1)
    b_kv, blk = c // 4, c % 4
    rows = slice(KVBLK * blk, KVBLK * blk + KVBLK)
    cosq2, sinq2 = _rope_tables_paired(sidx)
    cosk2, sink2 = _rope_tables_paired(
        np.arange(KVBLK * blk, KVBLK * blk + KVBLK))
    mask = np.zeros((128, MW), f)
    for (b, j), (o, mw) in mofs.items():
        W = Wt[b, j]
        kvp = np.arange(128 * j, 128 * j + 128)
        mask[:, o:o + mw] = (pos[b, None, W:W + mw] >= kvp[:, None])
    mask4 = np.repeat(mask, 4, axis=1)
    # head-norm reduction weights (gain and 1/64 folded) + sqrt biases
    gq = qg.astype(np.float64)
    ind2g = np.zeros((128, 2 * NH // 2), np.float64)
    biasq = np.zeros((2, NH // 2 + 1), np.float64)
    for m in range(NH // 2):
        for hh in range(2):
            g = gq[2 * m + hh]
            ind2g[64 * hh:64 * hh + 64, 2 * m + hh] = 1.0 / (g * g)
            biasq[hh, m] = 64.0 * EPS / (g * g)
    ind2k = np.zeros((128, 2), np.float64)
    ind2k[0:64, 0] = 1.0 / 64.0
    ind2k[64:128, 1] = 1.0 / 64.0
    biasq[:, NH // 2] = EPS
    ind2T = np.zeros((2, 128), np.float64)
    ind2T[0, 0:64] = 1.0
    ind2T[1, 64:128] = 1.0
    ones1024 = np.full((128, 1), 1.0 / 1024.0, np.float64)
    T = lambda a: np.ascontiguousarray(a.T.astype(f))
    return {
        "xqT": T(xm[bidx, sidx]),
        "xqbT": np.ascontiguousarray(xm[bidx, sidx].T.astype(bf)),
        "velqT": T(vel[bidx, sidx]),
        "xkvT": np.ascontiguousarray(xm[b_kv, rows].T.astype(bf)),
        "cqpT": cqpT, "ckpT": ckpT, "cv_wT": cv_wT, "proj_wT": proj_wT,
        "gu": np.ascontiguousarray(gate_up[c].astype(bf)),
        "dn": np.ascontiguousarray(down[c].astype(bf)),
        "ascalev": _vec8(attn_scale), "mscalev": _vec8(mlp_scale),
        "mucv": _vec8(mu_c),
        "ind2g": np.ascontiguousarray(ind2g.astype(bf)),
        "ind2k": np.ascontiguousarray(ind2k.astype(bf)),
        "ind2T": np.ascontiguousarray(ind2T.astype(f)),
        "ones1024": np.ascontiguousarray(ones1024.astype(bf)),
        "biasq": np.ascontiguousarray(biasq.astype(f)),
        "cosq2": cosq2, "sinq2": sinq2, "cosk2": cosk2, "sink2": sink2,
        "maskcat": np.ascontiguousarray(mask4.astype(bf)),
    }


_PROG_CACHE = {}


def _prep(inputs):
    f = np.float32
    bf = ml_dtypes.bfloat16
    x = np.asarray(inputs["x"], f)
    x0 = np.asarray(inputs["x0"], f)
    vel = np.asarray(inputs["vel"], f)
    rm = np.asarray(inputs["resid_mix"], f)
    xm = rm[0][None, None, :] * x + rm[1][None, None, :] * x0
    mu_c = np.clip(np.asarray(inputs["mu"], f), f(0.5), f(1.5)).astype(f)
    qg = np.asarray(inputs["q_gain"], f)
    cqpT = np.ascontiguousarray(
        np.asarray(inputs["cq_w"], f).T[:, PERMQ].astype(bf))
    ckpT = np.ascontiguousarray(
        np.asarray(inputs["ck_w"], f).T[:, PERMK].astype(bf))
    cv_wT = np.ascontiguousarray(np.asarray(inputs["cv_w"], f).T.astype(bf))
    proj_wT = np.ascontiguousarray(
        np.asarray(inputs["proj_w"], f).T.astype(bf))

    cols, nreal, CB = _route(inputs["eids"])
    Wt, Mt = _windows(cols, CB)
    groups = _groups4(Wt, CB)
    mofs, MW = _mask_layout(Wt, Mt, CB)
    meta = (cols, nreal, CB, Wt, Mt, groups, mofs, MW)
    in_maps = [
        _build_core_inputs(c, xm, vel,
                           np.asarray(inputs["attn_scale"], f),
                           np.asarray(inputs["mlp_scale"], f), mu_c, qg,
                           cqpT, ckpT, cv_wT, proj_wT,
                           np.asarray(inputs["gate_up"], f),
                           np.asarray(inputs["down"], f),
                           cols, CB, Wt, Mt, mofs, MW)
        for c in range(NCORES)
    ]
    return meta, in_maps


def _assemble(results, meta):
    f = np.float32
    cols, nreal, CB = meta[0], meta[1], meta[2]
    x_out = np.zeros((B, S, D), f)
    v_out = np.zeros((B, S, D), f)
    for c in range(NCORES):
        xoT = results[c]["xoutT"]
        vnT = results[c]["vnT"]
        for b in range(B):
            n = int(nreal[c, b])
            if n == 0:
                continue
            sl = slice(b * CB, b * CB + n)
            x_out[b, cols[c, b, :n]] = xoT[:, sl].T
            v_out[b, cols[c, b, :n]] = vnT[:, sl].T
    return x_out, v_out


def get_program(meta):
    cols, nreal, CB, Wt, Mt, groups, mofs, MW = meta
    key = (CB, MW, tuple(Wt.reshape(-1)), tuple(Mt.reshape(-1)))
    if key not in _PROG_CACHE:
        _PROG_CACHE[key] = build_program(CB, Wt, Mt, groups, mofs, MW)
    return _PROG_CACHE[key]


def kernel(**inputs):
    meta, in_maps = _prep(inputs)
    nc = get_program(meta)
    res = run_bass_kernel_spmd(nc, in_maps, core_ids=list(range(NCORES)))
    return _assemble(res.results, meta)


# ------------------------------------------------------------- device side

def _chunks(n, limit=512):
    return [(s, min(limit, n - s)) for s in range(0, n, limit)]


def _bank_splits(lo, hi, bank=512):
    """Split [lo, hi) at multiples of `bank`."""
    out, p = [], lo
    while p < hi:
        q = min(hi, (p // bank + 1) * bank)
        out.append((p, q - p))
        p = q
    return out


def build_program(CB, Wt, Mt, groups, mofs, MW, n_devices=NCORES):
    C = B * CB
    nc = bacc.Bacc("TRN2", target_bir_lowering=False, debug=False,
                   num_devices=n_devices)
    dt = F32
    d_in = {}
    for name, shape, dty in [
        ("xqT", [D, C], dt), ("xqbT", [D, C], BF16), ("velqT", [D, C], dt),
        ("xkvT", [D, KVBLK], BF16),
        ("ascalev", [128, NDT], dt), ("mscalev", [128, NDT], dt),
        ("mucv", [128, NDT], dt), ("biasq", [2, NH // 2 + 1], dt),
        ("ind2g", [128, NH], BF16), ("ind2k", [128, 2], BF16),
        ("ind2T", [2, 128], dt), ("ones1024", [128, 1], BF16),
        ("cosq2", [128, C], dt), ("sinq2", [128, C], dt),
        ("cosk2", [128, KVBLK], dt), ("sink2", [128, KVBLK], dt),
        ("maskcat", [128, 4 * MW], BF16),
        ("cqpT", [D, D], BF16), ("ckpT", [D, KVD], BF16),
        ("cv_wT", [D, KVD], BF16), ("proj_wT", [D, D], BF16),
        ("gu", [D, 2 * INTER], BF16), ("dn", [INTER, D], BF16),
    ]:
        d_in[name] = nc.dram_tensor(name, shape, dty, kind="ExternalInput")
    d_xout = nc.dram_tensor("xoutT", [D, C], dt, kind="ExternalOutput")
    d_vn = nc.dram_tensor("vnT", [D, C], dt, kind="ExternalOutput")

    with tile.TileContext(nc) as tc:
        _emit(tc, nc, d_in, d_xout, d_vn, CB, Wt, Mt, groups, mofs)
    nc.compile()
    return nc


def _emit(tc, nc, d_in, d_xout, d_vn, CB, Wt, Mt, groups, mofs):
    C = B * CB
    dt = F32
    sy, gp, ve, sc, pe = nc.sync, nc.gpsimd, nc.vector, nc.scalar, nc.tensor

    es = contextlib.ExitStack()
    cst = es.enter_context(tc.tile_pool(name="const", bufs=1))
    agd = es.enter_context(tc.tile_pool(name="agD", bufs=1, space="DRAM"))

    epsc = cst.tile([1, 1], dt, tag="epsc")
    ve.memset(epsc[:], EPS)
    vecs = {}
    for nm, shape, dty in (
            ("ascalev", [128, NDT], dt), ("mscalev", [128, NDT], dt),
            ("mucv", [128, NDT], dt), ("biasq", [2, NH // 2 + 1], dt),
            ("ind2g", [128, NH], BF16), ("ind2k", [128, 2], BF16),
            ("ind2T", [2, 128], dt), ("ones1024", [128, 1], BF16)):
        t = cst.tile(shape, dty, tag=nm, name=nm)
        sy.dma_start(t[:], d_in[nm].ap())
        vecs[nm] = t
    tbl = {}
    for nm, w in (("cosq2", C), ("sinq2", C), ("cosk2", KVBLK),
                  ("sink2", KVBLK)):
        t = cst.tile([128, w], dt, tag=nm, name=nm)
        sy.dma_start(t[:], d_in[nm].ap())
        tbl[nm] = t
    mask_sb = cst.tile([128, d_in["maskcat"].shape[1]], BF16, tag="mask")
    sy.dma_start(mask_sb[:], d_in["maskcat"].ap())

    agk_in = agd.tile([KVD, KVBLK], BF16, tag="agk_in")
    agv_in = agd.tile([KVBLK, KVD], BF16, tag="agv_in")
    agk_out = agd.tile([NCORES * KVD, KVBLK], BF16, addr_space="Shared",
                       tag="agk_out")
    agv_out = agd.tile([NCORES * KVBLK, KVD], BF16, addr_space="Shared",
                       tag="agv_out")

    def head_rope(rot, pstmp, psq, w, cosap, sinap, bidx, pbufs=2):
        """psq [128,w] PSUM (2 heads) -> (q1 bf16 SBUF, invb PSUM f32)."""
        sq = rot.tile([128, w], BF16, tag="hsq", name="hsq", bufs=3)
        sc.activation(sq[:], psq[:], ACT.Square)
        hs = pstmp.tile([2, w], dt, tag="hps", name="hps", bufs=pbufs)
        ind = vecs["ind2k"] if bidx == NH // 2 else vecs["ind2g"]
        isl = ind[:, 0:2] if bidx == NH // 2 else ind[:, 2 * bidx:2 * bidx + 2]
        pe.matmul(hs[:], isl, sq[:], start=True, stop=True)
        rt = rot.tile([2, w], dt, tag="hrt", name="hrt", bufs=2)
        sc.activation(rt[:], hs[:], ACT.Sqrt,
                      bias=vecs["biasq"][:, bidx:bidx + 1])
        rts = rot.tile([2, w], dt, tag="hrts", name="hrts", bufs=2)
        inv2 = rot.tile([2, w], dt, tag="hinv", name="hinv", bufs=2)
        ve.reciprocal_approx_accurate(inv2[:], rt[:], rts[:])
        invb = pstmp.tile([128, w], dt, tag="hinvb", name="hinvb", bufs=pbufs)
        pe.matmul(invb[:], vecs["ind2T"][:], inv2[:], start=True, stop=True)
        t1 = rot.tile([128, w], BF16, tag="ht1", name="ht1", bufs=2)
        ve.tensor_mul(t1[:], psq[:], cosap)
        sw = rot.tile([128, w], dt, tag="hsw", name="hsw", bufs=2)
        ve.stream_shuffle(sw[:], psq[:], SWAP_MASK)
        t2 = rot.tile([128, w], BF16, tag="ht2", name="ht2", bufs=2)
        ve.tensor_mul(t2[:], sw[:], sinap)
        q1 = rot.tile([128, w], BF16, tag="hq1", name="hq1", bufs=2)
        ve.tensor_add(q1[:], t1[:], t2[:])
        return q1, invb

    # ============================ Stage A: KV ============================
    with tc.tile_pool(name="kvA", bufs=1) as kva, \
         tc.tile_pool(name="kvR", bufs=2) as kvr, \
         tc.tile_pool(name="kvP", bufs=2, space="PSUM") as kvp:
        ckw, cvw, xkv = [], [], []
        for i in range(NDT):
            t = kva.tile([128, KVD], BF16, tag=f"ckw{i}", name=f"ckw{i}")
            sy.dma_start(t[:], d_in["ckpT"].ap()[128 * i:128 * (i + 1), :])
            ckw.append(t)
            t2 = kva.tile([128, KVD], BF16, tag=f"cvw{i}", name=f"cvw{i}")
            sy.dma_start(t2[:], d_in["cv_wT"].ap()[128 * i:128 * (i + 1), :])
            cvw.append(t2)
            t3 = kva.tile([128, KVBLK], BF16, tag=f"xkv{i}", name=f"xkv{i}")
            sy.dma_start(t3[:], d_in["xkvT"].ap()[128 * i:128 * (i + 1), :])
            xkv.append(t3)
        ssum = kvp.tile([1, KVBLK], dt, tag="nss", name="nss", bufs=1)
        for i in range(NDT):
            sq = kvr.tile([128, KVBLK], BF16, tag="nsq", name="nsq", bufs=3)
            sc.activation(sq[:], xkv[i][:], ACT.Square)
            pe.matmul(ssum[:], vecs["ones1024"][:], sq[:],
                      start=(i == 0), stop=(i == NDT - 1))
        nrt = kvr.tile([1, KVBLK], dt, tag="nrt", name="nrt", bufs=2)
        sc.activation(nrt[:], ssum[:], ACT.Sqrt, bias=epsc[0:1])
        nrs = kvr.tile([1, KVBLK], dt, tag="nrs", name="nrs", bufs=2)
        ninv = kvr.tile([1, KVBLK], dt, tag="ninv", name="ninv", bufs=2)
        ve.reciprocal_approx_accurate(ninv[:], nrt[:], nrs[:])
        nbc = kvr.tile([128, KVBLK], dt, tag="nbc", name="nbc", bufs=2)
        gp.partition_broadcast(nbc[:], ninv[0:1, :])
        nk = []
        for i in range(NDT):
            t = kva.tile([128, KVBLK], BF16, tag=f"nk{i}", name=f"nk{i}")
            ve.tensor_mul(t[:], xkv[i][:], nbc[:])
            nk.append(t)
        for mk in range(2):
            pk = kvp.tile([128, KVBLK], dt, tag="pkT", name="pkT", bufs=2)
            for i in range(NDT):
                pe.matmul(pk[:], ckw[i][:, 128 * mk:128 * (mk + 1)], nk[i][:],
                          start=(i == 0), stop=(i == NDT - 1))
            q1, invb = head_rope(kvr, kvp, pk, KVBLK,
                                 tbl["cosk2"][:], tbl["sink2"][:], NH // 2,
                                 pbufs=1)
            kro = kvr.tile([128, KVBLK], BF16, tag="kro", name="kro", bufs=2)
            ve.tensor_mul(kro[:], q1[:], invb[:])
            sy.dma_start(agk_in[128 * mk:128 * (mk + 1), :], kro[:])
        for m4 in range(4):
            pv = kvp.tile([128, KVD], dt, tag="pv", name="pv", bufs=2,
                          padded_shape=[128, 512])
            for i in range(NDT):
                pe.matmul(pv[:], nk[i][:, 128 * m4:128 * (m4 + 1)], cvw[i][:],
                          start=(i == 0), stop=(i == NDT - 1))
            vsb = kvr.tile([128, KVD], BF16, tag="vsb", name="vsb", bufs=2)
            ve.tensor_copy(vsb[:], pv[:])
            sy.dma_start(agv_in[128 * m4:128 * (m4 + 1), :], vsb[:])

    gp.collective_compute("AllGather", ALU.bypass,
                          replica_groups=[list(range(NCORES))],
                          ins=[agk_in.opt()], outs=[agk_out.opt()])
    gp.collective_compute("AllGather", ALU.bypass,
                          replica_groups=[list(range(NCORES))],
                          ins=[agv_in.opt()], outs=[agv_out.opt()])

    # ===================== Stage B1: Q proj/norm/rope ====================
    qa = es.enter_context(tc.tile_pool(name="qa", bufs=1))      # xmq f32
    yap = es.enter_context(tc.tile_pool(name="yap", bufs=1))    # yall
    qrp = es.enter_context(tc.tile_pool(name="qrop", bufs=1))   # qro4
    yall = [yap.tile([128, C], BF16, tag=f"yall{i}", name=f"yall{i}")
            for i in range(NDT)]
    qro4 = [qrp.tile([64, 4 * C], BF16, tag=f"qro4_{kh}", name=f"qro4_{kh}")
            for kh in range(NKV)]
    xmq = []
    with tc.tile_pool(name="qt", bufs=1) as qt, \
         tc.tile_pool(name="qrot", bufs=2) as qr2, \
         tc.tile_pool(name="qP", bufs=2, space="PSUM") as qp:
        xqb, cqw = [], []
        for i in range(NDT):
            xq = qa.tile([128, C], dt, tag=f"xmq{i}", name=f"xmq{i}")
            sy.dma_start(xq[:], d_in["xqT"].ap()[128 * i:128 * (i + 1), :])
            xmq.append(xq)
            t = qt.tile([128, C], BF16, tag=f"xqb{i}", name=f"xqb{i}")
            sy.dma_start(t[:], d_in["xqbT"].ap()[128 * i:128 * (i + 1), :])
            xqb.append(t)
            t2 = qt.tile([128, D], BF16, tag=f"cqw{i}", name=f"cqw{i}")
            sy.dma_start(t2[:], d_in["cqpT"].ap()[128 * i:128 * (i + 1), :])
            cqw.append(t2)
        for m in range(NH // 2):
            kh, p = m // 2, m % 2
            for (s, w) in _chunks(C):
                psq = qp.tile([128, w], dt, tag="psq", name="psq", bufs=2,
                              padded_shape=[128, 512])
                for i in range(NDT):
                    pe.matmul(psq[:], cqw[i][:, 128 * m:128 * (m + 1)],
                              xqb[i][:, s:s + w],
                              start=(i == 0), stop=(i == NDT - 1))
                q1, invb = head_rope(qr2, qp, psq, w,
                                     tbl["cosq2"][:, s:s + w],
                                     tbl["sinq2"][:, s:s + w], m)
                for hh in range(2):
                    st = 4 * s + 2 * p + hh
                    ve.tensor_mul(qro4[kh][:, st:st + 4 * (w - 1) + 1:4],
                                  q1[64 * hh:64 * hh + 64, :],
                                  invb[64 * hh:64 * hh + 64, :])

    # ============== Stage B2 + B3: attention + proj + PID, per b ==============
    es2 = contextlib.ExitStack()
    pj = es2.enter_context(tc.tile_pool(name="pj", bufs=1))
    pjw = []
    at = es2.enter_context(tc.tile_pool(name="at", bufs=1))
    atr = es2.enter_context(tc.tile_pool(name="atR", bufs=2))
    atp = es2.enter_context(tc.tile_pool(name="atP", bufs=1, space="PSUM"))
    pjp = es2.enter_context(tc.tile_pool(name="pjP", bufs=1, space="PSUM"))
    for b in range(B):
        kall = []
        for kh in range(NKV):
            t = at.tile([64, S], BF16, tag=f"kall{kh}", name=f"kall{kh}",
                        bufs=2)
            for r4 in range(4):
                r = 4 * b + r4
                sy.dma_start(t[:, 512 * r4:512 * (r4 + 1)],
                             agk_out[KVD * r + 64 * kh:
                                     KVD * r + 64 * kh + 64, :])
            kall.append(t)
        vx = {}
        for j in range(NKVT):
            if Wt[b, j] >= CB:
                continue
            r = 4 * b + j // 4
            loc = 128 * (j % 4)
            t = at.tile([128, 4 * 65], BF16, tag=f"vx{j}", name=f"vx{j}",
                        bufs=2)
            dst = t[:, 0:260].rearrange("p (k c) -> p k c", k=4, c=65)
            src = agv_out[KVBLK * r + loc:KVBLK * r + loc + 128, 0:KVD]
            sy.dma_start(dst[:, :, 0:64],
                         src.rearrange("p (k c) -> p k c", k=4, c=64))
            gp.memset(t[:, 64:260:65], 1.0)
            vx[j] = t
        for kh in range(NKV):
            pys = atp.tile([65, 4 * CB], dt, tag="pys", name="pys", bufs=1,
                           padded_shape=[65, 3 * 512])
            first, last = None, None
            plan = []
            for g in groups[b]:
                pvs = []
                for (j, t0, tl, ofs4) in g:
                    for (p0, pw) in _bank_splits(4 * t0, 4 * (t0 + tl)):
                        pvs.append((j, t0, ofs4, p0, pw))
                plan.append(pvs)
            flat = [x for pvs in plan for x in pvs]
            for gi, g in enumerate(groups[b]):
                gw4 = g[-1][3] + 4 * g[-1][2]
                sts = atp.tile([128, gw4], dt, tag="sts", name="sts", bufs=2,
                               padded_shape=[128, STRIP4])
                for (j, t0, tl, ofs4) in g:
                    for (p0, pw) in _bank_splits(ofs4, ofs4 + 4 * tl):
                        qcol = 4 * (b * CB + t0) + (p0 - ofs4)
                        pe.matmul(sts[:, p0:p0 + pw],
                                  kall[kh][:, 128 * j:128 * (j + 1)],
                                  qro4[kh][:, qcol:qcol + pw],
                                  start=True, stop=True)
                prb = atr.tile([128, gw4], BF16, tag="prb", name="prb",
                               bufs=2, padded_shape=[128, STRIP4])
                sc.activation(prb[:], sts[:], ACT.Exp)
                for (j, t0, tl, ofs4) in g:
                    if (b, j) not in mofs:
                        continue
                    mo, mw = mofs[(b, j)]
                    ms, me = Wt[b, j], Wt[b, j] + mw
                    a, e = max(ms, t0), min(me, t0 + tl)
                    if a >= e:
                        continue
                    pcol = ofs4 + 4 * (a - t0)
                    mcol = 4 * (mo + (a - ms))
                    gp.tensor_mul(prb[:, pcol:pcol + 4 * (e - a)],
                                  prb[:, pcol:pcol + 4 * (e - a)],
                                  mask_sb[:, mcol:mcol + 4 * (e - a)])
                for (j, t0, ofs4, p0, pw) in plan[gi]:
                    pe.matmul(pys[:, p0:p0 + pw],
                              vx[j][:, 65 * kh:65 * kh + 65],
                              prb[:, ofs4 + (p0 - 4 * t0):
                                  ofs4 + (p0 - 4 * t0) + pw],
                              start=(j == 0), stop=((j, t0, ofs4, p0, pw)
                                                    == flat[-1]),
                              skip_group_check=True)
            rc0 = atr.tile([1, 4 * CB], dt, tag="rc0", name="rc0", bufs=2)
            ve.tensor_copy(rc0[:], pys[64:65, :])
            rcs = atr.tile([1, 4 * CB], dt, tag="rcs", name="rcs", bufs=2)
            rc = atr.tile([1, 4 * CB], dt, tag="rc", name="rc", bufs=2)
            ve.reciprocal_approx_accurate(rc[:], rc0[:], rcs[:])
            yb = atr.tile([64, 4 * CB], dt, tag="yb", name="yb", bufs=2)
            gp.partition_broadcast(yb[:], rc[0:1, :])
            for u in range(4):
                h = 4 * kh + u
                ve.tensor_mul(
                    yall[h // 2][64 * (h % 2):64 * (h % 2) + 64,
                                 b * CB:b * CB + CB],
                    pys[0:64, u:4 * CB:4], yb[:, u:4 * CB:4])
        # ---------------- B3(b): out-proj + PID ----------------
        if not pjw:
            for i in range(NDT):
                t = pj.tile([128, D], BF16, tag=f"pjw{i}", name=f"pjw{i}")
                sy.dma_start(t[:],
                             d_in["proj_wT"].ap()[128 * i:128 * (i + 1), :])
                pjw.append(t)
        cb0 = b * CB
        for m in range(NDT):
            velm = atr.tile([128, CB], dt, tag="velm", name="velm", bufs=2)
            sy.dma_start(velm[:],
                         d_in["velqT"].ap()[128 * m:128 * (m + 1),
                                            cb0:cb0 + CB])
            pso = pjp.tile([128, CB], dt, tag="pso", name="pso", bufs=1,
                           padded_shape=[128, 512])
            for i in range(NDT):
                pe.matmul(pso[:], pjw[i][:, 128 * m:128 * (m + 1)],
                          yall[i][:, cb0:cb0 + CB],
                          start=(i == 0), stop=(i == NDT - 1))
            xb = xmq[m][:, cb0:cb0 + CB]
            ve.scalar_tensor_tensor(xb, pso[:], vecs["ascalev"][:, m:m + 1],
                                    xb, ALU.mult, ALU.add)
            t2 = atr.tile([128, CB], dt, tag="t2", name="t2", bufs=2)
            ve.tensor_scalar(t2[:], xb, vecs["mucv"][:, m:m + 1],
                             0.3, ALU.subtract, ALU.mult)
            vn = atr.tile([128, CB], dt, tag="vn", name="vn", bufs=2)
            ve.scalar_tensor_tensor(vn[:], velm[:], 0.95, t2[:],
                                    ALU.mult, ALU.subtract)
            ve.tensor_scalar(vn[:], vn[:], 3.0, -3.0, ALU.min, ALU.max)
            sy.dma_start(d_vn.ap()[128 * m:128 * (m + 1), cb0:cb0 + CB],
                         vn[:])
            ve.scalar_tensor_tensor(xb, vn[:], 0.1 * 0.1, xb,
                                    ALU.mult, ALU.add)
    es2.close()

    # ============================ Stage B4: MoE ============================
    with tc.tile_pool(name="mo", bufs=1) as mo, \
         tc.tile_pool(name="moR", bufs=2) as mor, \
         tc.tile_pool(name="moP", bufs=2, space="PSUM") as mop:
        guw, mn = [], []
        for i in range(NDT):
            t = mo.tile([128, 2 * INTER], BF16, tag=f"guw{i}", name=f"guw{i}")
            sy.dma_start(t[:], d_in["gu"].ap()[128 * i:128 * (i + 1), :])
            guw.append(t)
            mn.append(mo.tile([128, C], BF16, tag=f"mn{i}", name=f"mn{i}"))
        dnw = []
        for i2 in range(4):
            t = mo.tile([128, D], BF16, tag=f"dnw{i2}", name=f"dnw{i2}")
            sy.dma_start(t[:], d_in["dn"].ap()[128 * i2:128 * (i2 + 1), :])
            dnw.append(t)
        for (s, w) in _chunks(C):
            mss = mop.tile([1, w], dt, tag="mss", name="mss", bufs=2,
                           padded_shape=[1, 512])
            for i in range(NDT):
                msq = mor.tile([128, w], BF16, tag="msq", name="msq", bufs=3,
                               padded_shape=[128, 512])
                sc.activation(msq[:], xmq[i][:, s:s + w], ACT.Square)
                pe.matmul(mss[:], vecs["ones1024"][:], msq[:],
                          start=(i == 0), stop=(i == NDT - 1))
            mrt = mor.tile([1, w], dt, tag="mrt", name="mrt", bufs=2,
                           padded_shape=[1, 512])
            sc.activation(mrt[:], mss[:], ACT.Sqrt, bias=epsc[0:1])
            mrs = mor.tile([1, w], dt, tag="mrs", name="mrs", bufs=2,
                           padded_shape=[1, 512])
            minv = mor.tile([1, w], dt, tag="minv", name="minv", bufs=2,
                            padded_shape=[1, 512])
            ve.reciprocal_approx_accurate(minv[:], mrt[:], mrs[:])
            mbc = mor.tile([128, w], dt, tag="mbc", name="mbc", bufs=2,
                           padded_shape=[128, 512])
            gp.partition_broadcast(mbc[:], minv[0:1, :])
            for i in range(NDT):
                ve.tensor_mul(mn[i][:, s:s + w], xmq[i][:, s:s + w], mbc[:])
        sg, hh_t = [], []
        for m in range(NDT):
            for (s, w) in _chunks(C):
                psh = mop.tile([128, w], dt, tag="psh", name="psh", bufs=2,
                               padded_shape=[128, 512])
                for i in range(NDT):
                    pe.matmul(psh[:], guw[i][:, 128 * m:128 * (m + 1)],
                              mn[i][:, s:s + w],
                              start=(i == 0), stop=(i == NDT - 1))
                if m < 4:
                    if s == 0:
                        sgm = mo.tile([128, C], dt, tag=f"sg{m}",
                                      name=f"sg{m}")
                        sg.append(sgm)
                    sc.activation(sg[m][:, s:s + w], psh[:], ACT.Silu)
                else:
                    if s == 0:
                        hm = mo.tile([128, C], BF16, tag=f"hh{m - 4}",
                                     name=f"hh{m - 4}")
                        hh_t.append(hm)
                    ve.tensor_mul(hh_t[m - 4][:, s:s + w],
                                  sg[m - 4][:, s:s + w], psh[:])
        for m in range(NDT):
            xo = mor.tile([128, C], dt, tag="xo", name="xo", bufs=2)
            for (s, w) in _chunks(C):
                psm = mop.tile([128, w], dt, tag="psm", name="psm", bufs=2,
                               padded_shape=[128, 512])
                for i2 in range(4):
                    pe.matmul(psm[:], dnw[i2][:, 128 * m:128 * (m + 1)],
                              hh_t[i2][:, s:s + w],
                              start=(i2 == 0), stop=(i2 == 3))
                ve.scalar_tensor_tensor(xo[:, s:s + w], psm[:],
                                        vecs["mscalev"][:, m:m + 1],
                                        xmq[m][:, s:s + w],
                                        ALU.mult, ALU.add)
            sy.dma_start(d_xout.ap()[128 * m:128 * (m + 1), :], xo[:])

    es.close()


# revision 41
# speedup vs baseline: 1.3542x; 1.3542x over previous
"""Trainium2 Bass kernel for nn_Block_85598698209846 (moe_routing).

Strategy (8 NeuronCores, SPMD single program, per-core data):
- Tokens are assigned to cores BY EXPERT (host routes via eids): core c owns
  exactly the tokens that route to expert c, sorted by (batch, position).
  MoE then needs no communication and each core loads only its expert.
- Attention: K/V are computed in contiguous position blocks (core r owns
  block r) and shared via AllGather; each core computes Q for its
  scattered-but-sorted tokens. Causality is recovered with compile-time
  column windows (shared across cores) plus small per-core uploaded masks.
  Softmax runs without max-subtraction (|scores| <= 8), matching reference.
- The input rms_norm is skipped on the Q/K path (the per-head rms_norm is
  scale-invariant, so it cancels); V and the MoE input use the real norm.
- Rope uses a host-permuted "paired" dim layout so the rotation partner
  sits on the adjacent partition: one stream_shuffle replaces 4 copies.
- All 4 GQA query heads of one kv head are interleaved into a single
  score strip (columns 4t+u), quartering matmul/exp instruction counts.
- Layout: all activations transposed [D on partitions, tokens on free].
"""
import contextlib
import numpy as np
import ml_dtypes

import concourse.bass as bass
import concourse.bacc as bacc
import concourse.tile as tile
from concourse import mybir
from concourse.bass_utils import run_bass_kernel_spmd

B, S, D = 2, 2048, 1024
NH, NKV, HD = 16, 4, 64
KVD = NKV * HD
NE, INTER = 8, 512
EPS = float(np.float32(1.1920929e-07))
NCORES = 8
KVBLK = 512          # seq rows per core in the KV phase
NKVT = S // 128      # 16 kv tiles per batch
NDT = D // 128       # 8 d-tiles
F32 = mybir.dt.float32
BF16 = mybir.dt.bfloat16
STRIP4 = 1024        # score-strip width in interleaved cols (2 PSUM banks)
TOKCAP = STRIP4 // 4
ALU = mybir.AluOpType
ACT = mybir.ActivationFunctionType
SWAP_MASK = [x for k in range(16) for x in (2 * k + 1, 2 * k)]


# ---------------------------------------------------------------- host side

def _pair_perm(nheads):
    """Output-dim permutation putting rope partners on adjacent rows."""
    p = []
    for h in range(nheads):
        for i in range(HD // 2):
            p += [64 * h + i, 64 * h + 32 + i]
    return np.array(p, dtype=np.int64)


PERMQ = _pair_perm(NH)
PERMK = _pair_perm(NKV)


def _route(eids):
    eids = np.asarray(eids).astype(np.int64)
    lists = [[np.sort(np.where(eids[b] == e)[0]) for b in range(B)]
             for e in range(NE)]
    maxn = max(len(lists[e][b]) for e in range(NE) for b in range(B))
    CB = max(64, ((maxn + 63) // 64) * 64)
    cols = np.zeros((NE, B, CB), dtype=np.int64)
    nreal = np.zeros((NE, B), dtype=np.int64)
    for e in range(NE):
        for b in range(B):
            L = lists[e][b]
            nreal[e, b] = len(L)
            if len(L):
                cols[e, b, :len(L)] = L
                cols[e, b, len(L):] = L[-1]
    return cols, nreal, CB


def _windows(cols, CB):
    Wt = np.zeros((B, NKVT), dtype=np.int64)
    Mt = np.zeros((B, NKVT), dtype=np.int64)
    for b in range(B):
        for j in range(NKVT):
            Wt[b, j] = min(int(np.searchsorted(cols[e, b], 128 * j))
                           for e in range(NE))
            Mt[b, j] = max(int(np.searchsorted(cols[e, b], 128 * j + 127))
                           for e in range(NE))
    return Wt, Mt


def _groups4(Wt, CB):
    """Per batch: split each kv tile's token window into parts of <= TOKCAP
    tokens and first-fit-decreasing pack them into strips of <= STRIP4
    interleaved cols. groups[b] = list of strips; strip = list of
    (j, t0, tl, ofs4). Strips/parts are ordered so every j==0 part (which
    PSUM-initializes its column range) is emitted before any accumulate."""
    groups = []
    for b in range(B):
        parts = []
        for j in range(NKVT):
            t0 = int(Wt[b, j])
            while t0 < CB:
                tl = min(TOKCAP, CB - t0)
                parts.append((j, t0, tl))
                t0 += tl
        parts.sort(key=lambda p: -p[2])
        bins = []                     # [remaining, [(j,t0,tl,ofs4)...]]
        for (j, t0, tl) in parts:
            w4 = 4 * tl
            for bn in bins:
                if bn[0] >= w4:
                    bn[1].append((j, t0, tl, STRIP4 - bn[0]))
                    bn[0] -= w4
                    break
            else:
                bins.append([STRIP4 - w4, [(j, t0, tl, 0)]])
        for bn in bins:               # j==0 parts first within each strip
            bn[1].sort(key=lambda p: (p[0] != 0,))
        bins.sort(key=lambda bn: (not any(p[0] == 0 for p in bn[1]),))
        groups.append([bn[1] for bn in bins])
    return groups


def _mask_layout(Wt, Mt, CB):
    ofs, total = {}, 0
    for b in range(B):
        for j in range(NKVT):
            if Wt[b, j] >= CB:
                continue
            mw = int(min(Mt[b, j], CB) - Wt[b, j])
            if mw <= 0:
                continue
            ofs[(b, j)] = (total, mw)
            total += mw
    return ofs, max(total, 1)


def _rope_tables_paired(positions):
    """[128, n] cos2/sin2 for paired-layout rope (2 heads per tile)."""
    inv_freq = (1.0 / 10000.0 ** (np.arange(0, HD, 2, dtype=np.float32) / HD)
                ).astype(np.float32)
    fr = np.outer(inv_freq, positions.astype(np.float32))  # [32, n]
    c, s = np.cos(fr), np.sin(fr)
    half_c = np.empty((64, fr.shape[1]), np.float32)
    half_s = np.empty((64, fr.shape[1]), np.float32)
    half_c[0::2], half_c[1::2] = c, c
    half_s[0::2], half_s[1::2] = s, -s
    cos2 = np.concatenate([half_c, half_c], axis=0)
    sin2 = np.concatenate([half_s, half_s], axis=0)
    return np.ascontiguousarray(cos2), np.ascontiguousarray(sin2)


def _vec8(v):
    return np.ascontiguousarray(np.asarray(v, np.float32).reshape(NDT, 128).T)


def _build_core_inputs(c, xm, vel, attn_scale, mlp_scale, mu_c, qg,
                       cqpT, ckpT, cv_wT, proj_wT, gate_up, down,
                       cols, CB, Wt, Mt, mofs, MW):
    f = np.float32
    bf = ml_dtypes.bfloat16
    pos = cols[c]                                     # [B, CB]
    bidx = np.repeat(np.arange(B), CB)
    sidx = pos.reshape(-1)
    b_kv, blk = c // 4, c % 4
    rows = slice(KVBLK * blk, KVBLK * blk + KVBLK)
    cosq2, sinq2 = _rope_tables_paired(sidx)
    cosk2, sink2 = _rope_tables_paired(
        np.arange(KVBLK * blk, KVBLK * blk + KVBLK))
    mask = np.zeros((128, MW), f)
    for (b, j), (o, mw) in mofs.items():
        W = Wt[b, j]
        kvp = np.arange(128 * j, 128 * j + 128)
        mask[:, o:o + mw] = (pos[b, None, W:W + mw] >= kvp[:, None])
    mask4 = np.repeat(mask, 4, axis=1)
    # head-norm reduction weights (gain and 1/64 folded) + sqrt biases
    gq = qg.astype(np.float64)
    ind2g = np.zeros((128, 2 * NH // 2), np.float64)
    biasq = np.zeros((2, NH // 2 + 1), np.float64)
    for m in range(NH // 2):
        for hh in range(2):
            g = gq[2 * m + hh]
            ind2g[64 * hh:64 * hh + 64, 2 * m + hh] = 1.0 / (g * g)
            biasq[hh, m] = 64.0 * EPS / (g * g)
    ind2k = np.zeros((128, 2), np.float64)
    ind2k[0:64, 0] = 1.0 / 64.0
    ind2k[64:128, 1] = 1.0 / 64.0
    biasq[:, NH // 2] = EPS
    ind2T = np.zeros((2, 128), np.float64)
    ind2T[0, 0:64] = 1.0
    ind2T[1, 64:128] = 1.0
    ones1024 = np.full((128, 1), 1.0 / 1024.0, np.float64)
    T = lambda a: np.ascontiguousarray(a.T.astype(f))
    return {
        "xqT": T(xm[bidx, sidx]),
        "xqbT": np.ascontiguousarray(xm[bidx, sidx].T.astype(bf)),
        "velqT": T(vel[bidx, sidx]),
        "xkvT": np.ascontiguousarray(xm[b_kv, rows].T.astype(bf)),
        "cqpT": cqpT, "ckpT": ckpT, "cv_wT": cv_wT, "proj_wT": proj_wT,
        "gu": np.ascontiguousarray(gate_up[c].astype(bf)),
        "dn": np.ascontiguousarray(down[c].astype(bf)),
        "ascalev": _vec8(attn_scale), "mscalev": _vec8(mlp_scale),
        "mucv": _vec8(mu_c),
        "ind2g": np.ascontiguousarray(ind2g.astype(bf)),
        "ind2k": np.ascontiguousarray(ind2k.astype(bf)),
        "ind2T": np.ascontiguousarray(ind2T.astype(f)),
        "ones1024": np.ascontiguousarray(ones1024.astype(bf)),
        "biasq": np.ascontiguousarray(biasq.astype(f)),
        "cosq2": cosq2, "sinq2": sinq2, "cosk2": cosk2, "sink2": sink2,
        "maskcat": np.ascontiguousarray(mask4.astype(bf)),
    }


_PROG_CACHE = {}


def _prep(inputs):
    f = np.float32
    bf = ml_dtypes.bfloat16
    x = np.asarray(inputs["x"], f)
    x0 = np.asarray(inputs["x0"], f)
    vel = np.asarray(inputs["vel"], f)
    rm = np.asarray(inputs["resid_mix"], f)
    xm = rm[0][None, None, :] * x + rm[1][None, None, :] * x0
    mu_c = np.clip(np.asarray(inputs["mu"], f), f(0.5), f(1.5)).astype(f)
    qg = np.asarray(inputs["q_gain"], f)
    cqpT = np.ascontiguousarray(
        np.asarray(inputs["cq_w"], f).T[:, PERMQ].astype(bf))
    ckpT = np.ascontiguousarray(
        np.asarray(inputs["ck_w"], f).T[:, PERMK].astype(bf))
    cv_wT = np.ascontiguousarray(np.asarray(inputs["cv_w"], f).T.astype(bf))
    proj_wT = np.ascontiguousarray(
        np.asarray(inputs["proj_w"], f).T.astype(bf))

    cols, nreal, CB = _route(inputs["eids"])
    Wt, Mt = _windows(cols, CB)
    groups = _groups4(Wt, CB)
    mofs, MW = _mask_layout(Wt, Mt, CB)
    meta = (cols, nreal, CB, Wt, Mt, groups, mofs, MW)
    in_maps = [
        _build_core_inputs(c, xm, vel,
                           np.asarray(inputs["attn_scale"], f),
                           np.asarray(inputs["mlp_scale"], f), mu_c, qg,
                           cqpT, ckpT, cv_wT, proj_wT,
                           np.asarray(inputs["gate_up"], f),
                           np.asarray(inputs["down"], f),
                           cols, CB, Wt, Mt, mofs, MW)
        for c in range(NCORES)
    ]
    return meta, in_maps


def _assemble(results, meta):
    f = np.float32
    cols, nreal, CB = meta[0], meta[1], meta[2]
    x_out = np.zeros((B, S, D), f)
    v_out = np.zeros((B, S, D), f)
    for c in range(NCORES):
        xoT = results[c]["xoutT"]
        vnT = results[c]["vnT"]
        for b in range(B):
            n = int(nreal[c, b])
            if n == 0:
                continue
            sl = slice(b * CB, b * CB + n)
            x_out[b, cols[c, b, :n]] = xoT[:, sl].T
            v_out[b, cols[c, b, :n]] = vnT[:, sl].T
    return x_out, v_out


def get_program(meta):
    cols, nreal, CB, Wt, Mt, groups, mofs, MW = meta
    key = (CB, MW, tuple(Wt.reshape(-1)), tuple(Mt.reshape(-1)))
    if key not in _PROG_CACHE:
        _PROG_CACHE[key] = build_program(CB, Wt, Mt, groups, mofs, MW)
    return _PROG_CACHE[key]


def kernel(**inputs):
    meta, in_maps = _prep(inputs)
    nc = get_program(meta)
    res = run_bass_kernel_spmd(nc, in_maps, core_ids=list(range(NCORES)))
    return _assemble(res.results, meta)


# ------------------------------------------------------------- device side

def _chunks(n, limit=512):
    return [(s, min(limit, n - s)) for s in range(0, n, limit)]


def _bank_splits(lo, hi, bank=512):
    """Split [lo, hi) at multiples of `bank`."""
    out, p = [], lo
    while p < hi:
        q = min(hi, (p // bank + 1) * bank)
        out.append((p, q - p))
        p = q
    return out


def build_program(CB, Wt, Mt, groups, mofs, MW, n_devices=NCORES, dbg=False):
    C = B * CB
    nc = bacc.Bacc("TRN2", target_bir_lowering=False, debug=False,
                   num_devices=n_devices)
    dt = F32
    d_in = {}
    for name, shape, dty in [
        ("xqT", [D, C], dt), ("xqbT", [D, C], BF16), ("velqT", [D, C], dt),
        ("xkvT", [D, KVBLK], BF16),
        ("ascalev", [128, NDT], dt), ("mscalev", [128, NDT], dt),
        ("mucv", [128, NDT], dt), ("biasq", [2, NH // 2 + 1], dt),
        ("ind2g", [128, NH], BF16), ("ind2k", [128, 2], BF16),
        ("ind2T", [2, 128], dt), ("ones1024", [128, 1], BF16),
        ("cosq2", [128, C], dt), ("sinq2", [128, C], dt),
        ("cosk2", [128, KVBLK], dt), ("sink2", [128, KVBLK], dt),
        ("maskcat", [128, 4 * MW], BF16),
        ("cqpT", [D, D], BF16), ("ckpT", [D, KVD], BF16),
        ("cv_wT", [D, KVD], BF16), ("proj_wT", [D, D], BF16),
        ("gu", [D, 2 * INTER], BF16), ("dn", [INTER, D], BF16),
    ]:
        d_in[name] = nc.dram_tensor(name, shape, dty, kind="ExternalInput")
    d_xout = nc.dram_tensor("xoutT", [D, C], dt, kind="ExternalOutput")
    d_vn = nc.dram_tensor("vnT", [D, C], dt, kind="ExternalOutput")
    d_dbg = {}
    if dbg:
        for name, shape in [("dbg_qro", [4 * 64, 4 * C]),
                            ("dbg_yall", [D, C]), ("dbg_x3", [D, C]),
                            ("dbg_mn", [D, C]),
                            ("dbg_kall", [4 * 64, S]),
                            ("dbg_vx", [128, 16 * 260]),
                            ("dbg_ysb", [65, 4 * CB]),
                            ("dbg_yall_b0", [D, C]),
                            ("dbg_qro_post", [4 * 64, 4 * C])]:
            d_dbg[name] = nc.dram_tensor(name, shape, dt,
                                         kind="ExternalOutput")

    with tile.TileContext(nc) as tc:
        _emit(tc, nc, d_in, d_xout, d_vn, CB, Wt, Mt, groups, mofs, d_dbg)
    nc.compile()
    return nc


def _emit(tc, nc, d_in, d_xout, d_vn, CB, Wt, Mt, groups, mofs, d_dbg={}):
    C = B * CB
    dt = F32
    sy, gp, ve, sc, pe = nc.sync, nc.gpsimd, nc.vector, nc.scalar, nc.tensor

    es = contextlib.ExitStack()
    cst = es.enter_context(tc.tile_pool(name="const", bufs=1))
    agd = es.enter_context(tc.tile_pool(name="agD", bufs=1, space="DRAM"))

    epsc = cst.tile([1, 1], dt, tag="epsc")
    ve.memset(epsc[:], EPS)
    vecs = {}
    for nm, shape, dty in (
            ("ascalev", [128, NDT], dt), ("mscalev", [128, NDT], dt),
            ("mucv", [128, NDT], dt), ("biasq", [2, NH // 2 + 1], dt),
            ("ind2g", [128, NH], BF16), ("ind2k", [128, 2], BF16),
            ("ind2T", [2, 128], dt), ("ones1024", [128, 1], BF16)):
        t = cst.tile(shape, dty, tag=nm, name=nm)
        sy.dma_start(t[:], d_in[nm].ap())
        vecs[nm] = t
    tbl = {}
    for nm, w in (("cosq2", C), ("sinq2", C), ("cosk2", KVBLK),
                  ("sink2", KVBLK)):
        t = cst.tile([128, w], dt, tag=nm, name=nm)
        sy.dma_start(t[:], d_in[nm].ap())
        tbl[nm] = t
    mask_sb = cst.tile([128, d_in["maskcat"].shape[1]], BF16, tag="mask")
    sy.dma_start(mask_sb[:], d_in["maskcat"].ap())

    agk_in = agd.tile([KVD, KVBLK], BF16, tag="agk_in")
    agv_in = agd.tile([KVBLK, KVD], BF16, tag="agv_in")
    agk_out = agd.tile([NCORES * KVD, KVBLK], BF16, addr_space="Shared",
                       tag="agk_out")
    agv_out = agd.tile([NCORES * KVBLK, KVD], BF16, addr_space="Shared",
                       tag="agv_out")

    def head_rope(rot, pstmp, psq, w, cosap, sinap, bidx, pbufs=2):
        """psq [128,w] PSUM (2 heads) -> (q1 bf16 SBUF, invb PSUM f32)."""
        sq = rot.tile([128, w], BF16, tag="hsq", name="hsq", bufs=3)
        sc.activation(sq[:], psq[:], ACT.Square)
        hs = pstmp.tile([2, w], dt, tag="hps", name="hps", bufs=pbufs)
        ind = vecs["ind2k"] if bidx == NH // 2 else vecs["ind2g"]
        isl = ind[:, 0:2] if bidx == NH // 2 else ind[:, 2 * bidx:2 * bidx + 2]
        pe.matmul(hs[:], isl, sq[:], start=True, stop=True)
        rt = rot.tile([2, w], dt, tag="hrt", name="hrt", bufs=2)
        sc.activation(rt[:], hs[:], ACT.Sqrt,
                      bias=vecs["biasq"][:, bidx:bidx + 1])
        inv2 = rot.tile([2, w], dt, tag="hinv", name="hinv", bufs=2)
        ve.reciprocal_approx_fast(out=inv2[:], in_=rt[:])
        invb = pstmp.tile([128, w], dt, tag="hinvb", name="hinvb", bufs=pbufs)
        pe.matmul(invb[:], vecs["ind2T"][:], inv2[:], start=True, stop=True)
        t1 = rot.tile([128, w], BF16, tag="ht1", name="ht1", bufs=2)
        ve.tensor_mul(t1[:], psq[:], cosap)
        sw = rot.tile([128, w], dt, tag="hsw", name="hsw", bufs=2)
        ve.stream_shuffle(sw[:], psq[:], SWAP_MASK)
        t2 = rot.tile([128, w], BF16, tag="ht2", name="ht2", bufs=2)
        ve.tensor_mul(t2[:], sw[:], sinap)
        q1 = rot.tile([128, w], BF16, tag="hq1", name="hq1", bufs=2)
        ve.tensor_add(q1[:], t1[:], t2[:])
        return q1, invb

    # ============================ Stage A: KV ============================
    with tc.tile_pool(name="kvA", bufs=1) as kva, \
         tc.tile_pool(name="kvR", bufs=2) as kvr, \
         tc.tile_pool(name="kvP", bufs=2, space="PSUM") as kvp:
        ckw, cvw, xkv = [], [], []
        for i in range(NDT):
            t = kva.tile([128, KVD], BF16, tag=f"ckw{i}", name=f"ckw{i}")
            sy.dma_start(t[:], d_in["ckpT"].ap()[128 * i:128 * (i + 1), :])
            ckw.append(t)
            t2 = kva.tile([128, KVD], BF16, tag=f"cvw{i}", name=f"cvw{i}")
            sy.dma_start(t2[:], d_in["cv_wT"].ap()[128 * i:128 * (i + 1), :])
            cvw.append(t2)
            t3 = kva.tile([128, KVBLK], BF16, tag=f"xkv{i}", name=f"xkv{i}")
            sy.dma_start(t3[:], d_in["xkvT"].ap()[128 * i:128 * (i + 1), :])
            xkv.append(t3)
        ssum = kvp.tile([1, KVBLK], dt, tag="nss", name="nss", bufs=1)
        for i in range(NDT):
            sq = kvr.tile([128, KVBLK], BF16, tag="nsq", name="nsq", bufs=3)
            sc.activation(sq[:], xkv[i][:], ACT.Square)
            pe.matmul(ssum[:], vecs["ones1024"][:], sq[:],
                      start=(i == 0), stop=(i == NDT - 1))
        nrt = kvr.tile([1, KVBLK], dt, tag="nrt", name="nrt", bufs=2)
        sc.activation(nrt[:], ssum[:], ACT.Sqrt, bias=epsc[0:1])
        ninv = kvr.tile([1, KVBLK], dt, tag="ninv", name="ninv", bufs=2)
        ve.reciprocal_approx_fast(out=ninv[:], in_=nrt[:])
        nbc = kvr.tile([128, KVBLK], dt, tag="nbc", name="nbc", bufs=2)
        gp.partition_broadcast(nbc[:], ninv[0:1, :])
        nk = []
        for i in range(NDT):
            t = kva.tile([128, KVBLK], BF16, tag=f"nk{i}", name=f"nk{i}")
            ve.tensor_mul(t[:], xkv[i][:], nbc[:])
            nk.append(t)
        for mk in range(2):
            pk = kvp.tile([128, KVBLK], dt, tag="pkT", name="pkT", bufs=2)
            for i in range(NDT):
                pe.matmul(pk[:], ckw[i][:, 128 * mk:128 * (mk + 1)], nk[i][:],
                          start=(i == 0), stop=(i == NDT - 1))
            q1, invb = head_rope(kvr, kvp, pk, KVBLK,
                                 tbl["cosk2"][:], tbl["sink2"][:], NH // 2,
                                 pbufs=1)
            kro = kvr.tile([128, KVBLK], BF16, tag="kro", name="kro", bufs=2)
            ve.tensor_mul(kro[:], q1[:], invb[:])
            sy.dma_start(agk_in[128 * mk:128 * (mk + 1), :], kro[:])
        for m4 in range(4):
            pv = kvp.tile([128, KVD], dt, tag="pv", name="pv", bufs=2,
                          padded_shape=[128, 512])
            for i in range(NDT):
                pe.matmul(pv[:], nk[i][:, 128 * m4:128 * (m4 + 1)], cvw[i][:],
                          start=(i == 0), stop=(i == NDT - 1))
            vsb = kvr.tile([128, KVD], BF16, tag="vsb", name="vsb", bufs=2)
            ve.tensor_copy(vsb[:], pv[:])
            sy.dma_start(agv_in[128 * m4:128 * (m4 + 1), :], vsb[:])

    gp.collective_compute("AllGather", ALU.bypass,
                          replica_groups=[list(range(NCORES))],
                          ins=[agk_in.opt()], outs=[agk_out.opt()])
    gp.collective_compute("AllGather", ALU.bypass,
                          replica_groups=[list(range(NCORES))],
                          ins=[agv_in.opt()], outs=[agv_out.opt()])

    # ===================== Stage B1: Q proj/norm/rope ====================
    qa = es.enter_context(tc.tile_pool(name="qa", bufs=1))      # xmq f32
    yap = es.enter_context(tc.tile_pool(name="yap", bufs=1))    # yall
    qrp = es.enter_context(tc.tile_pool(name="qrop", bufs=1))   # qro4
    yall = [yap.tile([128, C], BF16, tag=f"yall{i}", name=f"yall{i}")
            for i in range(NDT)]
    qro4 = [qrp.tile([64, 4 * C], BF16, tag=f"qro4_{kh}", name=f"qro4_{kh}")
            for kh in range(NKV)]
    xmq = []
    with tc.tile_pool(name="qt", bufs=1) as qt, \
         tc.tile_pool(name="qrot", bufs=2) as qr2, \
         tc.tile_pool(name="qP", bufs=2, space="PSUM") as qp:
        xqb, cqw = [], []
        for i in range(NDT):
            xq = qa.tile([128, C], dt, tag=f"xmq{i}", name=f"xmq{i}")
            sy.dma_start(xq[:], d_in["xqT"].ap()[128 * i:128 * (i + 1), :])
            xmq.append(xq)
            t = qt.tile([128, C], BF16, tag=f"xqb{i}", name=f"xqb{i}")
            sy.dma_start(t[:], d_in["xqbT"].ap()[128 * i:128 * (i + 1), :])
            xqb.append(t)
            t2 = qt.tile([128, D], BF16, tag=f"cqw{i}", name=f"cqw{i}")
            sy.dma_start(t2[:], d_in["cqpT"].ap()[128 * i:128 * (i + 1), :])
            cqw.append(t2)
        for m in range(NH // 2):
            kh, p = m // 2, m % 2
            for (s, w) in _chunks(C):
                psq = qp.tile([128, w], dt, tag="psq", name="psq", bufs=2,
                              padded_shape=[128, 512])
                for i in range(NDT):
                    pe.matmul(psq[:], cqw[i][:, 128 * m:128 * (m + 1)],
                              xqb[i][:, s:s + w],
                              start=(i == 0), stop=(i == NDT - 1))
                q1, invb = head_rope(qr2, qp, psq, w,
                                     tbl["cosq2"][:, s:s + w],
                                     tbl["sinq2"][:, s:s + w], m)
                for hh in range(2):
                    st = 4 * s + 2 * p + hh
                    ve.tensor_mul(qro4[kh][:, st:st + 4 * (w - 1) + 1:4],
                                  q1[64 * hh:64 * hh + 64, :],
                                  invb[64 * hh:64 * hh + 64, :])

    if d_dbg:
        for kh in range(NKV):
            gp.dma_start(d_dbg["dbg_qro"].ap()[64 * kh:64 * (kh + 1), :],
                         qro4[kh][:])

    # ============== Stage B2 + B3: attention + proj + PID, per b ==============
    es2 = contextlib.ExitStack()
    pj = es2.enter_context(tc.tile_pool(name="pj", bufs=1))
    pjw = []
    at = es2.enter_context(tc.tile_pool(name="at", bufs=1))
    atr = es2.enter_context(tc.tile_pool(name="atR", bufs=2))
    atp = es2.enter_context(tc.tile_pool(name="atP", bufs=1, space="PSUM"))
    pjp = es2.enter_context(tc.tile_pool(name="pjP", bufs=1, space="PSUM"))
    for b in range(B):
        kall = []
        for kh in range(NKV):
            t = at.tile([64, S], BF16, tag=f"kall{kh}", name=f"kall{kh}",
                        bufs=2)
            for r4 in range(4):
                r = 4 * b + r4
                sy.dma_start(t[:, 512 * r4:512 * (r4 + 1)],
                             agk_out[KVD * r + 64 * kh:
                                     KVD * r + 64 * kh + 64, :])
            kall.append(t)
        vx = {}
        for j in range(NKVT):
            if Wt[b, j] >= CB:
                continue
            r = 4 * b + j // 4
            loc = 128 * (j % 4)
            t = at.tile([128, 4 * 65], BF16, tag=f"vx{j}", name=f"vx{j}",
                        bufs=2)
            dst = t[:, 0:260].rearrange("p (k c) -> p k c", k=4, c=65)
            src = agv_out[KVBLK * r + loc:KVBLK * r + loc + 128, 0:KVD]
            sy.dma_start(dst[:, :, 0:64],
                         src.rearrange("p (k c) -> p k c", k=4, c=64))
            gp.memset(t[:, 64:260:65], 1.0)
            vx[j] = t
        if d_dbg and b == 0:
            for kh in range(NKV):
                gp.dma_start(d_dbg["dbg_kall"].ap()[64 * kh:64 * (kh + 1), :],
                             kall[kh][:])
        for kh in range(NKV):
            pys = atp.tile([65, 4 * CB], dt, tag="pys", name="pys", bufs=1,
                           padded_shape=[65, 3 * 512])
            first, last = None, None
            plan = []
            for g in groups[b]:
                pvs = []
                for (j, t0, tl, ofs4) in g:
                    for (p0, pw) in _bank_splits(4 * t0, 4 * (t0 + tl)):
                        pvs.append((j, t0, ofs4, p0, pw))
                plan.append(pvs)
            flat = [x for pvs in plan for x in pvs]
            for gi, g in enumerate(groups[b]):
                gw4 = max(ofs4 + 4 * tl for (_, _, tl, ofs4) in g)
                sts = atp.tile([128, gw4], dt, tag="sts", name="sts", bufs=2,
                               padded_shape=[128, STRIP4])
                for (j, t0, tl, ofs4) in g:
                    for (p0, pw) in _bank_splits(ofs4, ofs4 + 4 * tl):
                        qcol = 4 * (b * CB + t0) + (p0 - ofs4)
                        pe.matmul(sts[:, p0:p0 + pw],
                                  kall[kh][:, 128 * j:128 * (j + 1)],
                                  qro4[kh][:, qcol:qcol + pw],
                                  start=True, stop=True)
                prb = atr.tile([128, gw4], BF16, tag="prb", name="prb",
                               bufs=2, padded_shape=[128, STRIP4])
                sc.activation(prb[:], sts[:], ACT.Exp)
                for (j, t0, tl, ofs4) in g:
                    if (b, j) not in mofs:
                        continue
                    mo, mw = mofs[(b, j)]
                    ms, me = Wt[b, j], Wt[b, j] + mw
                    a, e = max(ms, t0), min(me, t0 + tl)
                    if a >= e:
                        continue
                    pcol = ofs4 + 4 * (a - t0)
                    mcol = 4 * (mo + (a - ms))
                    ve.tensor_mul(prb[:, pcol:pcol + 4 * (e - a)],
                                  prb[:, pcol:pcol + 4 * (e - a)],
                                  mask_sb[:, mcol:mcol + 4 * (e - a)])
                for (j, t0, ofs4, p0, pw) in plan[gi]:
                    pe.matmul(pys[:, p0:p0 + pw],
                              vx[j][:, 65 * kh:65 * kh + 65],
                              prb[:, ofs4 + (p0 - 4 * t0):
                                  ofs4 + (p0 - 4 * t0) + pw],
                              start=(j == 0), stop=((j, t0, ofs4, p0, pw)
                                                    == flat[-1]),
                              skip_group_check=True)
            ysb = atr.tile([64, 4 * CB], dt, tag="ysb", name="ysb", bufs=2)
            ve.tensor_copy(ysb[:], pys[0:64, :])
            den = atr.tile([1, 4 * CB], dt, tag="den", name="den", bufs=2)
            ve.tensor_copy(den[:], pys[64:65, :])
            rc = atr.tile([1, 4 * CB], dt, tag="rc", name="rc", bufs=2)
            ve.reciprocal_approx_fast(out=rc[:], in_=den[:])
            yb = atr.tile([64, 4 * CB], dt, tag="yb", name="yb", bufs=2)
            gp.partition_broadcast(yb[:], rc[0:1, :])
            for u in range(4):
                h = 4 * kh + u
                ve.tensor_mul(
                    yall[h // 2][64 * (h % 2):64 * (h % 2) + 64,
                                 b * CB:b * CB + CB],
                    ysb[:, u:4 * CB:4], yb[:, u:4 * CB:4])
        if d_dbg and b == 0:
            for i in range(NDT):
                gp.dma_start(
                    d_dbg["dbg_yall_b0"].ap()[128 * i:128 * (i + 1), :],
                    yall[i][:])
            for kh in range(NKV):
                gp.dma_start(
                    d_dbg["dbg_qro_post"].ap()[64 * kh:64 * (kh + 1), :],
                    qro4[kh][:])
        # ---------------- B3(b): out-proj + PID ----------------
        if not pjw:
            for i in range(NDT):
                t = pj.tile([128, D], BF16, tag=f"pjw{i}", name=f"pjw{i}")
                sy.dma_start(t[:],
                             d_in["proj_wT"].ap()[128 * i:128 * (i + 1), :])
                pjw.append(t)
        cb0 = b * CB
        for m in range(NDT):
            velm = atr.tile([128, CB], dt, tag="velm", name="velm", bufs=2)
            sy.dma_start(velm[:],
                         d_in["velqT"].ap()[128 * m:128 * (m + 1),
                                            cb0:cb0 + CB])
            pso = pjp.tile([128, CB], dt, tag="pso", name="pso", bufs=1,
                           padded_shape=[128, 512])
            for i in range(NDT):
                pe.matmul(pso[:], pjw[i][:, 128 * m:128 * (m + 1)],
                          yall[i][:, cb0:cb0 + CB],
                          start=(i == 0), stop=(i == NDT - 1))
            xb = xmq[m][:, cb0:cb0 + CB]
            ve.scalar_tensor_tensor(xb, pso[:], vecs["ascalev"][:, m:m + 1],
                                    xb, ALU.mult, ALU.add)
            t2 = atr.tile([128, CB], dt, tag="t2", name="t2", bufs=2)
            ve.tensor_scalar(t2[:], xb, vecs["mucv"][:, m:m + 1],
                             0.3, ALU.subtract, ALU.mult)
            vn = atr.tile([128, CB], dt, tag="vn", name="vn", bufs=2)
            ve.scalar_tensor_tensor(vn[:], velm[:], 0.95, t2[:],
                                    ALU.mult, ALU.subtract)
            ve.tensor_scalar(vn[:], vn[:], 3.0, -3.0, ALU.min, ALU.max)
            sy.dma_start(d_vn.ap()[128 * m:128 * (m + 1), cb0:cb0 + CB],
                         vn[:])
            ve.scalar_tensor_tensor(xb, vn[:], 0.1 * 0.1, xb,
                                    ALU.mult, ALU.add)
    if d_dbg:
        for i in range(NDT):
            gp.dma_start(d_dbg["dbg_yall"].ap()[128 * i:128 * (i + 1), :],
                         yall[i][:])
            sy.dma_start(d_dbg["dbg_x3"].ap()[128 * i:128 * (i + 1), :],
                         xmq[i][:])
    es2.close()

    # ============================ Stage B4: MoE ============================
    with tc.tile_pool(name="mo", bufs=1) as mo, \
         tc.tile_pool(name="moR", bufs=2) as mor, \
         tc.tile_pool(name="moP", bufs=2, space="PSUM") as mop:
        guw, mn = [], []
        for i in range(NDT):
            t = mo.tile([128, 2 * INTER], BF16, tag=f"guw{i}", name=f"guw{i}")
            sy.dma_start(t[:], d_in["gu"].ap()[128 * i:128 * (i + 1), :])
            guw.append(t)
            mn.append(mo.tile([128, C], BF16, tag=f"mn{i}", name=f"mn{i}"))
        dnw = []
        for i2 in range(4):
            t = mo.tile([128, D], BF16, tag=f"dnw{i2}", name=f"dnw{i2}")
            sy.dma_start(t[:], d_in["dn"].ap()[128 * i2:128 * (i2 + 1), :])
            dnw.append(t)
        for (s, w) in _chunks(C):
            mss = mop.tile([1, w], dt, tag="mss", name="mss", bufs=2,
                           padded_shape=[1, 512])
            for i in range(NDT):
                msq = mor.tile([128, w], BF16, tag="msq", name="msq", bufs=3,
                               padded_shape=[128, 512])
                sc.activation(msq[:], xmq[i][:, s:s + w], ACT.Square)
                pe.matmul(mss[:], vecs["ones1024"][:], msq[:],
                          start=(i == 0), stop=(i == NDT - 1))
            mrt = mor.tile([1, w], dt, tag="mrt", name="mrt", bufs=2,
                           padded_shape=[1, 512])
            sc.activation(mrt[:], mss[:], ACT.Sqrt, bias=epsc[0:1])
            minv = mor.tile([1, w], dt, tag="minv", name="minv", bufs=2,
                            padded_shape=[1, 512])
            ve.reciprocal_approx_fast(out=minv[:], in_=mrt[:])
            mbc = mor.tile([128, w], dt, tag="mbc", name="mbc", bufs=2,
                           padded_shape=[128, 512])
            gp.partition_broadcast(mbc[:], minv[0:1, :])
            for i in range(NDT):
                ve.tensor_mul(mn[i][:, s:s + w], xmq[i][:, s:s + w], mbc[:])
        if d_dbg:
            for i in range(NDT):
                gp.dma_start(d_dbg["dbg_mn"].ap()[128 * i:128 * (i + 1), :],
                             mn[i][:])
        sg, hh_t = [], []
        for m in range(NDT):
            for (s, w) in _chunks(C):
                psh = mop.tile([128, w], dt, tag="psh", name="psh", bufs=2,
                               padded_shape=[128, 512])
                for i in range(NDT):
                    pe.matmul(psh[:], guw[i][:, 128 * m:128 * (m + 1)],
                              mn[i][:, s:s + w],
                              start=(i == 0), stop=(i == NDT - 1))
                if m < 4:
                    if s == 0:
                        sgm = mo.tile([128, C], dt, tag=f"sg{m}",
                                      name=f"sg{m}")
                        sg.append(sgm)
                    sc.activation(sg[m][:, s:s + w], psh[:], ACT.Silu)
                else:
                    if s == 0:
                        hm = mo.tile([128, C], BF16, tag=f"hh{m - 4}",
                                     name=f"hh{m - 4}")
                        hh_t.append(hm)
                    ve.tensor_mul(hh_t[m - 4][:, s:s + w],
                                  sg[m - 4][:, s:s + w], psh[:])
        for m in range(NDT):
            xo = mor.tile([128, C], dt, tag="xo", name="xo", bufs=2)
            for (s, w) in _chunks(C):
                psm = mop.tile([128, w], dt, tag="psm", name="psm", bufs=2,
                               padded_shape=[128, 512])
                for i2 in range(4):
                    pe.matmul(psm[:], dnw[i2][:, 128 * m:128 * (m + 1)],
                              hh_t[i2][:, s:s + w],
                              start=(i2 == 0), stop=(i2 == 3))
                ve.scalar_tensor_tensor(xo[:, s:s + w], psm[:],
                                        vecs["mscalev"][:, m:m + 1],
                                        xmq[m][:, s:s + w],
                                        ALU.mult, ALU.add)
            sy.dma_start(d_xout.ap()[128 * m:128 * (m + 1), :], xo[:])

    es.close()
